# revision 20
# baseline (speedup 1.0000x reference)
"""Attention graph convolution (GAT layer) on 8 TRN2 NeuronCores.

Reference computation (all fp32):
    h   = input @ W                      # (N, 64)
    e   = leakyrelu(h@a1 + (h@a2).T)     # (N, N)
    att = softmax(where(adj>0, e, -inf)) # row softmax
    out = elu(att @ h)                   # (N, 64)

Sharding: rows of e/att (= output rows) are split across 8 cores,
1536 rows each.  h (N x 64) is computed on every core (tiny).

Per-core algorithm (core owns rows I, |I| = 1536):
  - no max-subtraction softmax: z values are small (|z| < ~30), so
    U[i,j] = adj[i,j] * exp(leakyrelu(Wh1_i + Wh2_j)) cannot overflow and
    equals the reference numerator up to the common exp(-max) factor.
  - denominator via ones-column: P = U @ [h | 1]; out = elu(P[:, :64] / P[:, 64])
  - U is built in TRANSPOSED layout [j partitions, i free] so it can feed
    the PE matmul (contraction dim = partition dim) with no U transpose:
        P.T[f, i] = sum_j h_ext[j, f] * U.T[j, i]
    adj row-blocks are DMA'd contiguously (int32 -> bf16 cast in SWDGE,
    exact for 0/1) and transposed 128x128-at-a-time on the tensor engine
    into PSUM; the mask multiply reads adj.T directly from PSUM.
  - h/Wh1/Wh2 production (phase 1) is interleaved with the first window
    of the main loop so it overlaps the adjacency DMA stream.
"""

import numpy as np

N_TOTAL = 12288
K_IN = 128
F_OUT = 64
N_CORES = 8
ALPHA = 0.2


def build_program(
    nt: int,          # total nodes (columns of adj)
    no: int,          # nodes owned by this core (rows of adj block)
    jw: int,          # j window size (columns resident in SBUF at once)
    u_bf16: bool = False,  # U / h_ext in bf16 for the big matmul
    lrelu_act_frac: float = 0.60,  # j-chunk fraction with leakyrelu on ACT
):
    from contextlib import ExitStack

    import concourse.bass as bass
    import concourse.mybir as mybir
    import concourse.tile as tile
    from concourse import bacc
    from concourse.alu_op_type import AluOpType
    from concourse.masks import make_identity

    f32 = mybir.dt.float32
    i32 = mybir.dt.int32
    bf16 = mybir.dt.bfloat16
    AF = mybir.ActivationFunctionType
    u_dt = bf16 if u_bf16 else f32

    P = 128
    F = F_OUT
    FE = F + 1                    # h columns + ones column
    K = K_IN
    assert nt % P == 0 and no % P == 0 and jw % P == 0 and nt % jw == 0
    ncj = nt // P                 # global j chunks
    nw = nt // jw                 # windows
    cpw = jw // P                 # j chunks per window
    nic = no // P                 # i chunks (own rows)
    S = 512                       # i split for matmul N-dim / psum banks
    ns = (no + S - 1) // S
    assert no % S == 0 or ns == 1

    nc = bacc.Bacc("TRN2", target_bir_lowering=False, debug=False,
                   num_devices=1)

    inp = nc.dram_tensor("input", [nt, K], f32, kind="ExternalInput")
    inp_own = nc.dram_tensor("input_own", [no, K], f32, kind="ExternalInput")
    adj_own = nc.dram_tensor("adj_own", [no, nt], i32, kind="ExternalInput")
    w_d = nc.dram_tensor("W", [K, F], f32, kind="ExternalInput")
    a_d = nc.dram_tensor("a", [2 * F, 1], f32, kind="ExternalInput")
    out_d = nc.dram_tensor("out", [no, F], f32, kind="ExternalOutput")

    with tile.TileContext(nc) as tc, ExitStack() as ctx:
        consts = ctx.enter_context(tc.tile_pool(name="consts", bufs=1))

        identity = consts.tile([P, P], f32)
        make_identity(nc, identity)
        identity_bf = consts.tile([P, P], bf16)
        nc.vector.tensor_copy(identity_bf[:], identity[:])

        # shared small-psum scratch (phases 0/1/3); 1 bank — PSUM budget is
        # 4 (adjT double-buffered) + 3 (P.T accumulator) + 1 = 8 banks.
        scr_ps = ctx.enter_context(
            tc.tile_pool(name="scr_ps", bufs=1, space="PSUM"))

        # ---- phase 0: Wa1 = W @ a1, Wa2 = W @ a2 -------------------------
        wt_sb = consts.tile([F, K], f32)       # W.T  (64 x 128)
        nc.sync.dma_start(wt_sb[:], w_d.ap().rearrange("k f -> f k"))
        a_sb = consts.tile([F, 2], f32)        # [a1 | a2] (64 x 2)
        nc.sync.dma_start(a_sb[:], a_d.ap().rearrange("(n f) o -> f (n o)", n=2))
        wwa2_sb = consts.tile([K, FE], f32)    # [W | Wa2] (128 x 65)
        nc.sync.dma_start(wwa2_sb[:, 0:F], w_d.ap())

        wa12_sb = consts.tile([K, 2], f32)
        ones_sb = consts.tile([P, P], f32)
        nc.vector.memset(ones_sb[:], 1.0)
        wa1_rep = consts.tile([K, P], f32)     # Wa1 replicated to 128 cols

        wa_ps = scr_ps.tile([K, 2], f32, tag="scr")
        nc.tensor.matmul(wa_ps[:], wt_sb[:], a_sb[:], start=True, stop=True)
        nc.vector.tensor_copy(wa12_sb[:], wa_ps[:])
        nc.vector.tensor_copy(wwa2_sb[:, F:FE], wa12_sb[:, 1:2])
        # wa1_rep[k, m] = Wa1[k] for all m
        nc.vector.tensor_scalar(wa1_rep[:], ones_sb[:], wa12_sb[:, 0:1], None,
                                AluOpType.mult)

        # ---- phase 1a: Wh1_rep[p, x] = Wh1[own x] for all p --------------
        # Wh1_rep = wa1_rep.T @ input_own.T ; input_own.T via strided DMA.
        wh1_rep = consts.tile([P, no], f32)
        into_sb = consts.tile([K, no], f32)    # input_own.T
        nc.sync.dma_start(into_sb[:], inp_own.ap().rearrange("i k -> k i"))
        for s in range(ns):
            sw = min(S, no - s * S)
            w1p = scr_ps.tile([P, S], f32, tag="scr")
            nc.tensor.matmul(w1p[:, 0:sw], wa1_rep[:],
                             into_sb[:, s * S:s * S + sw],
                             start=True, stop=True)
            nc.vector.tensor_copy(wh1_rep[:, s * S:s * S + sw], w1p[:, 0:sw])

        # ---- phase 1b (emitted interleaved below): h_ext, Wh2 ------------
        h_ext = consts.tile([P, ncj, FE], u_dt)
        wh2_sb = consts.tile([P, ncj], f32)
        nc.vector.memset(h_ext[:, :, F], 1.0)

        in_t = ctx.enter_context(tc.tile_pool(name="in_t", bufs=4))

        def phase1b_chunk(jc):
            # input[jc].T via strided DMA; h_ext[:, jc, :] = [h | Wh2-col]
            jts = in_t.tile([K, P], f32, tag="jts")
            nc.sync.dma_start(
                jts[:], inp[jc * P:(jc + 1) * P, :].rearrange("j k -> k j"))
            hw_ps = scr_ps.tile([P, FE], f32, tag="scr")
            nc.tensor.matmul(hw_ps[:], jts[:], wwa2_sb[:],
                             start=True, stop=True)
            nc.scalar.copy(h_ext[:, jc, 0:F], hw_ps[:, 0:F])
            nc.vector.tensor_copy(wh2_sb[:, jc:jc + 1], hw_ps[:, F:FE])

        # ---- phase 2: main loop over j windows / j chunks ----------------
        pt_pool = ctx.enter_context(
            tc.tile_pool(name="pt_acc", bufs=1, space="PSUM"))
        pt_ps = pt_pool.tile([FE, no], f32)

        n_act = int(round(lrelu_act_frac * ncj))

        def lrelu_engine(jc):
            # deterministic interleave of ACT / DVE chunks
            return "act" if (jc * 7919) % ncj < n_act else "dve"

        def lrelu_chunk(jc, dst):
            if lrelu_engine(jc) == "act":
                nc.scalar.activation(dst, wh1_rep[:], AF.Prelu,
                                     bias=wh2_sb[:, jc:jc + 1],
                                     scale=1.0, alpha=ALPHA)
            else:
                # t = 0.2 * (Wh1 + Wh2) ; E = max(Wh1 + Wh2, t)
                nc.vector.tensor_scalar(dst, wh1_rep[:],
                                        wh2_sb[:, jc:jc + 1], ALPHA,
                                        AluOpType.add, AluOpType.mult)
                nc.vector.scalar_tensor_tensor(
                    dst, wh1_rep[:], wh2_sb[:, jc:jc + 1], dst,
                    AluOpType.add, AluOpType.max)

        with (
            tc.tile_pool(name="adjw", bufs=2 * nic) as adjw_pool,
            tc.tile_pool(name="adjt", bufs=2, space="PSUM") as tr_pool,
            tc.tile_pool(name="epool", bufs=2) as e_pool,
            tc.tile_pool(name="upool", bufs=2) as u_pool,
        ):
            adjw = {}
            assert cpw % 2 == 0
            for w in range(nw):
                # adj window DMA (SWDGE cast int32 -> bf16), one per i chunk
                for ic in range(nic):
                    t = adjw_pool.tile([P, jw], bf16, tag="adjw",
                                       name=f"adjw_{w}_{ic}")
                    nc.gpsimd.dma_start(
                        t[:],
                        adj_own[ic * P:(ic + 1) * P, w * jw:(w + 1) * jw])
                    adjw[ic] = t
                for jp in range(cpw // 2):
                    jcs = [w * cpw + 2 * jp, w * cpw + 2 * jp + 1]
                    if w == 0:
                        # interleave h/Wh2 production with the first window
                        for jc in range(jcs[0] * (ncj // cpw),
                                        (jcs[1] + 1) * (ncj // cpw)):
                            phase1b_chunk(jc)
                    # leakyrelu for both chunks, one batched exp
                    e_sb = e_pool.tile([P, 2, no], f32, tag="e")
                    for q, jc in enumerate(jcs):
                        lrelu_chunk(jc, e_sb[:, q, :])
                    nc.scalar.activation(e_sb[:], e_sb[:], AF.Exp)
                    for q, jc in enumerate(jcs):
                        jcl = jc - w * cpw
                        # adj.T for this j chunk (12 PE transposes -> PSUM)
                        at_ps = tr_pool.tile([P, no], bf16, tag="adjt")
                        for ic in range(nic):
                            nc.tensor.transpose(
                                at_ps[:, ic * P:(ic + 1) * P],
                                adjw[ic][:, jcl * P:(jcl + 1) * P],
                                identity_bf[:])
                        # U = E * adj.T, then accumulate P.T += h_ext.T @ U
                        u_sb = u_pool.tile([P, no], u_dt, tag="u")
                        nc.vector.tensor_tensor(
                            u_sb[:], e_sb[:, q, :], at_ps[:],
                            AluOpType.mult)
                        for s in range(ns):
                            sw = min(S, no - s * S)
                            nc.tensor.matmul(pt_ps[:, s * S:s * S + sw],
                                             h_ext[:, jc, :],
                                             u_sb[:, s * S:s * S + sw],
                                             start=(jc == 0),
                                             stop=(jc == ncj - 1))

        # ---- phase 3: out = elu(P[:, :64] / P[:, 64]) --------------------
        pt_sb = consts.tile([FE, no], f32)
        nc.vector.tensor_copy(pt_sb[:], pt_ps[:])
        out_sb = consts.tile([P, nic, F], f32)
        with tc.tile_pool(name="fin_sb", bufs=4) as fin_sb:
            for ic in range(nic):
                ptp = scr_ps.tile([P, FE], f32, tag="scr")
                nc.tensor.transpose(ptp[:], pt_sb[:, ic * P:(ic + 1) * P],
                                    identity[0:FE, 0:FE])
                rec = fin_sb.tile([P, 1], f32, tag="rec")
                nc.vector.reciprocal(rec[:], ptp[:, F:FE])
                hp = fin_sb.tile([P, F], f32, tag="hp")
                nc.vector.tensor_scalar(hp[:], ptp[:, 0:F], rec[:], None,
                                        AluOpType.mult)
                # elu(x) = max(x,0) + exp(min(x,0)) - 1
                mn = fin_sb.tile([P, F], f32, tag="mn")
                nc.vector.tensor_scalar(mn[:], hp[:], 0.0, None, AluOpType.min)
                nc.scalar.activation(mn[:], mn[:], AF.Exp)
                nc.vector.tensor_scalar(hp[:], hp[:], 0.0, None, AluOpType.max)
                nc.vector.scalar_tensor_tensor(
                    out_sb[:, ic, :], mn[:], 1.0, hp[:],
                    AluOpType.subtract, AluOpType.add)
        nc.sync.dma_start(out_d.ap().rearrange("(c p) f -> p c f", p=P),
                          out_sb[:])

    nc.compile()
    return nc


_CACHE = {}


def _get_program(nt, no, jw, **kw):
    key = (nt, no, jw, tuple(sorted(kw.items())))
    if key not in _CACHE:
        _CACHE[key] = build_program(nt, no, jw, **kw)
    return _CACHE[key]


def kernel(input, adj, W, a):
    from concourse.bass_utils import run_bass_kernel_spmd

    input = np.ascontiguousarray(input, dtype=np.float32)
    adj = np.ascontiguousarray(adj, dtype=np.int32)
    W = np.ascontiguousarray(W, dtype=np.float32)
    a = np.ascontiguousarray(a, dtype=np.float32)

    nt = input.shape[0]
    no = nt // N_CORES
    nc = _get_program(nt, no, 2048)

    in_maps = []
    for c in range(N_CORES):
        in_maps.append({
            "input": input,
            "input_own": input[c * no:(c + 1) * no],
            "adj_own": adj[c * no:(c + 1) * no],
            "W": W,
            "a": a,
        })
    res = run_bass_kernel_spmd(nc, in_maps, list(range(N_CORES)))
    return np.concatenate([r["out"] for r in res.results], axis=0)


# revision 22
# speedup vs baseline: 3.2267x; 3.2267x over previous
"""Attention graph convolution (GAT layer) on 8 TRN2 NeuronCores.

Reference computation (all fp32):
    h   = input @ W                      # (N, 64)
    e   = leakyrelu(h@a1 + (h@a2).T)     # (N, N)
    att = softmax(where(adj>0, e, -inf)) # row softmax
    out = elu(att @ h)                   # (N, 64)

Sharding: rows of e/att (= output rows) are split across 8 cores,
1536 rows each.  h (N x 64) is computed on every core (tiny).

Per-core algorithm (core owns rows I, |I| = 1536):
  - no max-subtraction softmax: z values are small (|z| < ~30), so
    U[i,j] = adj[i,j] * exp(leakyrelu(Wh1_i + Wh2_j)) cannot overflow and
    equals the reference numerator up to the common exp(-max) factor.
  - denominator via ones-column: P = U @ [h | 1]; out = elu(P[:, :64] / P[:, 64])
  - U is built in TRANSPOSED layout [j partitions, i free] so it can feed
    the PE matmul (contraction dim = partition dim) with no U transpose:
        P.T[f, i] = sum_j h_ext[j, f] * U.T[j, i]
    adj row-blocks are DMA'd contiguously (int32 -> bf16 cast in SWDGE,
    exact for 0/1) and transposed 128x128-at-a-time on the tensor engine
    into PSUM; the mask multiply reads adj.T directly from PSUM.
  - h/Wh1/Wh2 production (phase 1) is interleaved with the first window
    of the main loop so it overlaps the adjacency DMA stream.
"""

import numpy as np

N_TOTAL = 12288
K_IN = 128
F_OUT = 64
N_CORES = 8
ALPHA = 0.2


def build_program(
    nt: int,          # total nodes (columns of adj)
    no: int,          # nodes owned by this core (rows of adj block)
    jw: int,          # j window size (columns resident in SBUF at once)
    u_bf16: bool = False,  # U / h_ext in bf16 for the big matmul
    lrelu_act_frac: float = 0.60,  # j-chunk fraction with leakyrelu on ACT
):
    from contextlib import ExitStack

    import concourse.bass as bass
    import concourse.mybir as mybir
    import concourse.tile as tile
    from concourse import bacc
    from concourse.alu_op_type import AluOpType
    from concourse.masks import make_identity

    f32 = mybir.dt.float32
    i32 = mybir.dt.int32
    bf16 = mybir.dt.bfloat16
    AF = mybir.ActivationFunctionType
    u_dt = bf16 if u_bf16 else f32

    P = 128
    F = F_OUT
    FE = F + 1                    # h columns + ones column
    K = K_IN
    assert nt % P == 0 and no % P == 0 and jw % P == 0 and nt % jw == 0
    ncj = nt // P                 # global j chunks
    nw = nt // jw                 # windows
    cpw = jw // P                 # j chunks per window
    nic = no // P                 # i chunks (own rows)
    S = 512                       # i split for matmul N-dim / psum banks
    ns = (no + S - 1) // S
    assert no % S == 0 or ns == 1

    nc = bacc.Bacc("TRN2", target_bir_lowering=False, debug=False,
                   num_devices=1)

    inp = nc.dram_tensor("input", [nt, K], f32, kind="ExternalInput")
    inp_own = nc.dram_tensor("input_own", [no, K], f32, kind="ExternalInput")
    adj_own = nc.dram_tensor("adj_own", [no, nt], i32, kind="ExternalInput")
    w_d = nc.dram_tensor("W", [K, F], f32, kind="ExternalInput")
    a_d = nc.dram_tensor("a", [2 * F, 1], f32, kind="ExternalInput")
    out_d = nc.dram_tensor("out", [no, F], f32, kind="ExternalOutput")

    with tile.TileContext(nc) as tc, ExitStack() as ctx:
        consts = ctx.enter_context(tc.tile_pool(name="consts", bufs=1))

        identity = consts.tile([P, P], f32)
        make_identity(nc, identity)
        identity_bf = consts.tile([P, P], bf16)
        nc.vector.tensor_copy(identity_bf[:], identity[:])

        # shared small-psum scratch (phases 0/1/3); 1 bank — PSUM budget is
        # 4 (adjT double-buffered) + 3 (P.T accumulator) + 1 = 8 banks.
        scr_ps = ctx.enter_context(
            tc.tile_pool(name="scr_ps", bufs=1, space="PSUM"))

        # ---- phase 0: Wa1 = W @ a1, Wa2 = W @ a2 -------------------------
        wt_sb = consts.tile([F, K], f32)       # W.T  (64 x 128)
        nc.sync.dma_start(wt_sb[:], w_d.ap().rearrange("k f -> f k"))
        a_sb = consts.tile([F, 2], f32)        # [a1 | a2] (64 x 2)
        nc.sync.dma_start(a_sb[:], a_d.ap().rearrange("(n f) o -> f (n o)", n=2))
        wwa2_sb = consts.tile([K, FE], f32)    # [W | Wa2] (128 x 65)
        nc.sync.dma_start(wwa2_sb[:, 0:F], w_d.ap())

        wa12_sb = consts.tile([K, 2], f32)
        ones_sb = consts.tile([P, P], f32)
        nc.vector.memset(ones_sb[:], 1.0)
        wa1_rep = consts.tile([K, P], f32)     # Wa1 replicated to 128 cols

        wa_ps = scr_ps.tile([K, 2], f32, tag="scr")
        nc.tensor.matmul(wa_ps[:], wt_sb[:], a_sb[:], start=True, stop=True)
        nc.vector.tensor_copy(wa12_sb[:], wa_ps[:])
        nc.vector.tensor_copy(wwa2_sb[:, F:FE], wa12_sb[:, 1:2])
        # wa1_rep[k, m] = Wa1[k] for all m
        nc.vector.tensor_scalar(wa1_rep[:], ones_sb[:], wa12_sb[:, 0:1], None,
                                AluOpType.mult)

        # ---- phase 1a: Wh1_rep[p, x] = Wh1[own x] for all p --------------
        # Wh1_rep = wa1_rep.T @ input_own.T ; input_own.T via PE transposes.
        wh1_rep = consts.tile([P, no], f32)
        into_sb = consts.tile([K, no], f32)    # input_own.T
        in_t = ctx.enter_context(tc.tile_pool(name="in_t", bufs=4))
        for ic in range(nic):
            ich = in_t.tile([P, K], f32, tag="ich")
            nc.sync.dma_start(ich[:], inp_own[ic * P:(ic + 1) * P, :])
            itp = scr_ps.tile([K, P], f32, tag="scr")
            nc.tensor.transpose(itp[:], ich[:], identity[:])
            nc.vector.tensor_copy(into_sb[:, ic * P:(ic + 1) * P], itp[:])
        for s in range(ns):
            sw = min(S, no - s * S)
            w1p = scr_ps.tile([P, S], f32, tag="scr")
            nc.tensor.matmul(w1p[:, 0:sw], wa1_rep[:],
                             into_sb[:, s * S:s * S + sw],
                             start=True, stop=True)
            nc.vector.tensor_copy(wh1_rep[:, s * S:s * S + sw], w1p[:, 0:sw])

        # ---- phase 1b (emitted interleaved below): h_ext, Wh2 ------------
        h_ext = consts.tile([P, ncj, FE], u_dt)
        wh2_sb = consts.tile([P, ncj], f32)
        nc.vector.memset(h_ext[:, :, F], 1.0)

        def phase1b_chunk(jc):
            # input[jc].T via PE transpose; h_ext[:, jc, :] = [h | Wh2-col]
            jch = in_t.tile([P, K], f32, tag="ich")
            nc.sync.dma_start(jch[:], inp[jc * P:(jc + 1) * P, :])
            jtp = scr_ps.tile([K, P], f32, tag="scr")
            nc.tensor.transpose(jtp[:], jch[:], identity[:])
            jts = in_t.tile([K, P], f32, tag="jts")
            nc.vector.tensor_copy(jts[:], jtp[:])
            hw_ps = scr_ps.tile([P, FE], f32, tag="scr")
            nc.tensor.matmul(hw_ps[:], jts[:], wwa2_sb[:],
                             start=True, stop=True)
            nc.scalar.copy(h_ext[:, jc, 0:F], hw_ps[:, 0:F])
            nc.vector.tensor_copy(wh2_sb[:, jc:jc + 1], hw_ps[:, F:FE])

        # ---- phase 2: main loop over j windows / j chunks ----------------
        pt_pool = ctx.enter_context(
            tc.tile_pool(name="pt_acc", bufs=1, space="PSUM"))
        pt_ps = pt_pool.tile([FE, no], f32)

        n_act = int(round(lrelu_act_frac * ncj))

        def lrelu_engine(jc):
            # deterministic interleave of ACT / DVE chunks
            return "act" if (jc * 7919) % ncj < n_act else "dve"

        def lrelu_chunk(jc, dst):
            if lrelu_engine(jc) == "act":
                nc.scalar.activation(dst, wh1_rep[:], AF.Prelu,
                                     bias=wh2_sb[:, jc:jc + 1],
                                     scale=1.0, alpha=ALPHA)
            else:
                # t = 0.2 * (Wh1 + Wh2) ; E = max(Wh1 + Wh2, t)
                nc.vector.tensor_scalar(dst, wh1_rep[:],
                                        wh2_sb[:, jc:jc + 1], ALPHA,
                                        AluOpType.add, AluOpType.mult)
                nc.vector.scalar_tensor_tensor(
                    dst, wh1_rep[:], wh2_sb[:, jc:jc + 1], dst,
                    AluOpType.add, AluOpType.max)

        # phase-1b chunks are produced spread over the first nw-1 windows,
        # always ahead of their first use by the main loop.
        per_w = -(-ncj // max(1, nw - 1))
        npair = cpw // 2
        assert cpw % 2 == 0

        with (
            tc.tile_pool(name="adjw", bufs=2 * nic) as adjw_pool,
            tc.tile_pool(name="adjt", bufs=4, space="PSUM") as tr_pool,
            tc.tile_pool(name="epool", bufs=2) as e_pool,
            tc.tile_pool(name="upool", bufs=2 * ns) as u_pool,
        ):
            adjw = {}
            for w in range(nw):
                # adj window DMA (SWDGE cast int32 -> bf16), one per i chunk
                for ic in range(nic):
                    t = adjw_pool.tile([P, jw], bf16, tag="adjw",
                                       name=f"adjw_{w}_{ic}")
                    nc.gpsimd.dma_start(
                        t[:],
                        adj_own[ic * P:(ic + 1) * P, w * jw:(w + 1) * jw])
                    adjw[ic] = t
                wchunks = list(range(w * per_w, min(ncj, (w + 1) * per_w)))
                for jp in range(npair):
                    for k, jc1b in enumerate(wchunks):
                        if k * npair // len(wchunks) == jp:
                            phase1b_chunk(jc1b)
                    jcs = [w * cpw + 2 * jp, w * cpw + 2 * jp + 1]
                    # leakyrelu for both chunks, one batched exp
                    e_sb = e_pool.tile([P, 2, no], f32, tag="e")
                    for q, jc in enumerate(jcs):
                        lrelu_chunk(jc, e_sb[:, q, :])
                    nc.scalar.activation(e_sb[:], e_sb[:], AF.Exp)
                    for q, jc in enumerate(jcs):
                        jcl = jc - w * cpw
                        for s in range(ns):
                            sw = min(S, no - s * S)
                            # adj.T for this i-split: regular bf16 matmuls
                            # against the identity (exact for 0/1) — gets
                            # FWL + weight double-buffering, unlike the
                            # transpose_mode path.
                            at_ps = tr_pool.tile([P, S], f32, tag="adjt")
                            for q2 in range(sw // P):
                                ic = (s * S) // P + q2
                                nc.tensor.matmul(
                                    at_ps[:, q2 * P:(q2 + 1) * P],
                                    adjw[ic][:, jcl * P:(jcl + 1) * P],
                                    identity_bf[:], start=True, stop=True)
                            # U = E * adj.T ; P.T += h_ext.T @ U
                            u_sb = u_pool.tile([P, S], u_dt, tag="u")
                            nc.vector.tensor_tensor(
                                u_sb[:, 0:sw],
                                e_sb[:, q, s * S:s * S + sw],
                                at_ps[:, 0:sw], AluOpType.mult)
                            nc.tensor.matmul(pt_ps[:, s * S:s * S + sw],
                                             h_ext[:, jc, :],
                                             u_sb[:, 0:sw],
                                             start=(jc == 0),
                                             stop=(jc == ncj - 1))

        # ---- phase 3: out = elu(P[:, :64] / P[:, 64]) --------------------
        pt_sb = consts.tile([FE, no], f32)
        nc.vector.tensor_copy(pt_sb[:], pt_ps[:])
        out_sb = consts.tile([P, nic, F], f32)
        with tc.tile_pool(name="fin_sb", bufs=4) as fin_sb:
            for ic in range(nic):
                ptp = scr_ps.tile([P, FE], f32, tag="scr")
                nc.tensor.transpose(ptp[:], pt_sb[:, ic * P:(ic + 1) * P],
                                    identity[0:FE, 0:FE])
                rec = fin_sb.tile([P, 1], f32, tag="rec")
                nc.vector.reciprocal(rec[:], ptp[:, F:FE])
                hp = fin_sb.tile([P, F], f32, tag="hp")
                nc.vector.tensor_scalar(hp[:], ptp[:, 0:F], rec[:], None,
                                        AluOpType.mult)
                # elu(x) = max(x,0) + exp(min(x,0)) - 1
                mn = fin_sb.tile([P, F], f32, tag="mn")
                nc.vector.tensor_scalar(mn[:], hp[:], 0.0, None, AluOpType.min)
                nc.scalar.activation(mn[:], mn[:], AF.Exp)
                nc.vector.tensor_scalar(hp[:], hp[:], 0.0, None, AluOpType.max)
                nc.vector.scalar_tensor_tensor(
                    out_sb[:, ic, :], mn[:], 1.0, hp[:],
                    AluOpType.subtract, AluOpType.add)
        nc.sync.dma_start(out_d.ap().rearrange("(c p) f -> p c f", p=P),
                          out_sb[:])

    nc.compile()
    return nc


_CACHE = {}


def _get_program(nt, no, jw, **kw):
    key = (nt, no, jw, tuple(sorted(kw.items())))
    if key not in _CACHE:
        _CACHE[key] = build_program(nt, no, jw, **kw)
    return _CACHE[key]


def kernel(input, adj, W, a):
    from concourse.bass_utils import run_bass_kernel_spmd

    input = np.ascontiguousarray(input, dtype=np.float32)
    adj = np.ascontiguousarray(adj, dtype=np.int32)
    W = np.ascontiguousarray(W, dtype=np.float32)
    a = np.ascontiguousarray(a, dtype=np.float32)

    nt = input.shape[0]
    no = nt // N_CORES
    nc = _get_program(nt, no, 2048)

    in_maps = []
    for c in range(N_CORES):
        in_maps.append({
            "input": input,
            "input_own": input[c * no:(c + 1) * no],
            "adj_own": adj[c * no:(c + 1) * no],
            "W": W,
            "a": a,
        })
    res = run_bass_kernel_spmd(nc, in_maps, list(range(N_CORES)))
    return np.concatenate([r["out"] for r in res.results], axis=0)


# revision 25
# speedup vs baseline: 3.3165x; 1.0278x over previous
"""Attention graph convolution (GAT layer) on 8 TRN2 NeuronCores.

Reference computation (all fp32):
    h   = input @ W                      # (N, 64)
    e   = leakyrelu(h@a1 + (h@a2).T)     # (N, N)
    att = softmax(where(adj>0, e, -inf)) # row softmax
    out = elu(att @ h)                   # (N, 64)

Sharding: rows of e/att (= output rows) are split across 8 cores,
1536 rows each.  h (N x 64) is computed on every core (tiny).

Per-core algorithm (core owns rows I, |I| = 1536):
  - no max-subtraction softmax: z values are small (|z| < ~30), so
    U[i,j] = adj[i,j] * exp(leakyrelu(Wh1_i + Wh2_j)) cannot overflow and
    equals the reference numerator up to the common exp(-max) factor.
  - denominator via ones-column: P = U @ [h | 1]; out = elu(P[:, :64] / P[:, 64])
  - U is built in TRANSPOSED layout [j partitions, i free] so it can feed
    the PE matmul (contraction dim = partition dim) with no U transpose:
        P.T[f, i] = sum_j h_ext[j, f] * U.T[j, i]
    adj row-blocks are DMA'd contiguously (int32 -> bf16 cast in SWDGE,
    exact for 0/1) and transposed 128x128-at-a-time on the tensor engine
    into PSUM; the mask multiply reads adj.T directly from PSUM.
  - h/Wh1/Wh2 production (phase 1) is interleaved with the first window
    of the main loop so it overlaps the adjacency DMA stream.
"""

import numpy as np

N_TOTAL = 12288
K_IN = 128
F_OUT = 64
N_CORES = 8
ALPHA = 0.2


def build_program(
    nt: int,          # total nodes (columns of adj)
    no: int,          # nodes owned by this core (rows of adj block)
    jw: int,          # j window size (columns resident in SBUF at once)
    u_bf16: bool = False,  # U / h_ext in bf16 for the big matmul
    lrelu_act_frac: float = 0.70,  # j-chunk fraction with leakyrelu on ACT
):
    from contextlib import ExitStack

    import concourse.bass as bass
    import concourse.mybir as mybir
    import concourse.tile as tile
    from concourse import bacc
    from concourse.alu_op_type import AluOpType
    from concourse.masks import make_identity

    f32 = mybir.dt.float32
    i32 = mybir.dt.int32
    bf16 = mybir.dt.bfloat16
    AF = mybir.ActivationFunctionType
    u_dt = bf16 if u_bf16 else f32

    P = 128
    F = F_OUT
    FE = F + 1                    # h columns + ones column
    K = K_IN
    assert nt % P == 0 and no % P == 0 and jw % P == 0 and nt % jw == 0
    ncj = nt // P                 # global j chunks
    nw = nt // jw                 # windows
    cpw = jw // P                 # j chunks per window
    nic = no // P                 # i chunks (own rows)
    S = 512                       # i split for matmul N-dim / psum banks
    ns = (no + S - 1) // S
    assert no % S == 0 or ns == 1

    nc = bacc.Bacc("TRN2", target_bir_lowering=False, debug=False,
                   num_devices=1)

    inp = nc.dram_tensor("input", [nt, K], f32, kind="ExternalInput")
    inp_own = nc.dram_tensor("input_own", [no, K], f32, kind="ExternalInput")
    adj_own = nc.dram_tensor("adj_own", [no, nt], i32, kind="ExternalInput")
    w_d = nc.dram_tensor("W", [K, F], f32, kind="ExternalInput")
    a_d = nc.dram_tensor("a", [2 * F, 1], f32, kind="ExternalInput")
    out_d = nc.dram_tensor("out", [no, F], f32, kind="ExternalOutput")

    with tile.TileContext(nc) as tc, ExitStack() as ctx:
        consts = ctx.enter_context(tc.tile_pool(name="consts", bufs=1))

        identity = consts.tile([P, P], f32)
        make_identity(nc, identity)
        identity_bf = consts.tile([P, P], bf16)
        nc.vector.tensor_copy(identity_bf[:], identity[:])

        # shared small-psum scratch (phases 0/1/3); 1 bank — PSUM budget is
        # 4 (adjT double-buffered) + 3 (P.T accumulator) + 1 = 8 banks.
        scr_ps = ctx.enter_context(
            tc.tile_pool(name="scr_ps", bufs=1, space="PSUM"))

        # ---- phase 0: Wa1 = W @ a1, Wa2 = W @ a2 -------------------------
        # (no strided DRAM reads: a 4-byte-element transposed W read costs
        # ~71 us of HWDGE descriptor generation and stalls the Sync queue)
        wwa2_sb = consts.tile([K, FE], f32)    # [W | Wa2] (128 x 65)
        nc.sync.dma_start(wwa2_sb[:, 0:F], w_d.ap())
        a_row = consts.tile([1, 2 * F], f32)   # a as a single-partition row
        nc.sync.dma_start(a_row[:], a_d.ap().rearrange("n o -> o n"))

        ones_sb = consts.tile([P, P], f32)
        nc.vector.memset(ones_sb[:], 1.0)
        # replicate a across partitions via a K=1 matmul with a ones row
        a_rep = consts.tile([P, 2 * F], f32)
        a_rep_ps = scr_ps.tile([P, 2 * F], f32, tag="scr")
        nc.tensor.matmul(a_rep_ps[:], ones_sb[0:1, :], a_row[:],
                         start=True, stop=True)
        nc.vector.tensor_copy(a_rep[:], a_rep_ps[:])

        wa12_sb = consts.tile([K, 2], f32)
        wtmp = consts.tile([K, F], f32)
        nc.vector.tensor_tensor(wtmp[:], wwa2_sb[:, 0:F], a_rep[:, 0:F],
                                AluOpType.mult)
        nc.vector.tensor_reduce(wa12_sb[:, 0:1], wtmp[:],
                                mybir.AxisListType.X, AluOpType.add)
        nc.vector.tensor_tensor(wtmp[:], wwa2_sb[:, 0:F], a_rep[:, F:2 * F],
                                AluOpType.mult)
        nc.vector.tensor_reduce(wa12_sb[:, 1:2], wtmp[:],
                                mybir.AxisListType.X, AluOpType.add)
        nc.vector.tensor_copy(wwa2_sb[:, F:FE], wa12_sb[:, 1:2])
        wa1_rep = consts.tile([K, P], f32)     # Wa1 replicated to 128 cols
        nc.vector.tensor_scalar(wa1_rep[:], ones_sb[:], wa12_sb[:, 0:1], None,
                                AluOpType.mult)

        # ---- phase 1a: Wh1_rep[p, x] = Wh1[own x] for all p --------------
        # Wh1_rep = wa1_rep.T @ input_own.T ; input_own.T via PE transposes.
        wh1_rep = consts.tile([P, no], f32)
        into_sb = consts.tile([K, no], f32)    # input_own.T
        in_t = ctx.enter_context(tc.tile_pool(name="in_t", bufs=4))
        for ic in range(nic):
            ich = in_t.tile([P, K], f32, tag="ich")
            nc.sync.dma_start(ich[:], inp_own[ic * P:(ic + 1) * P, :])
            itp = scr_ps.tile([K, P], f32, tag="scr")
            nc.tensor.transpose(itp[:], ich[:], identity[:])
            nc.vector.tensor_copy(into_sb[:, ic * P:(ic + 1) * P], itp[:])
        for s in range(ns):
            sw = min(S, no - s * S)
            w1p = scr_ps.tile([P, S], f32, tag="scr")
            nc.tensor.matmul(w1p[:, 0:sw], wa1_rep[:],
                             into_sb[:, s * S:s * S + sw],
                             start=True, stop=True)
            nc.vector.tensor_copy(wh1_rep[:, s * S:s * S + sw], w1p[:, 0:sw])

        # ---- phase 1b (emitted interleaved below): h_ext, Wh2 ------------
        h_ext = consts.tile([P, ncj, FE], u_dt)
        wh2_sb = consts.tile([P, ncj], f32)
        nc.vector.memset(h_ext[:, :, F], 1.0)

        def phase1b_chunk(jc):
            # input[jc].T via PE transpose; h_ext[:, jc, :] = [h | Wh2-col]
            jch = in_t.tile([P, K], f32, tag="ich")
            nc.sync.dma_start(jch[:], inp[jc * P:(jc + 1) * P, :])
            jtp = scr_ps.tile([K, P], f32, tag="scr")
            nc.tensor.transpose(jtp[:], jch[:], identity[:])
            jts = in_t.tile([K, P], f32, tag="jts")
            nc.vector.tensor_copy(jts[:], jtp[:])
            hw_ps = scr_ps.tile([P, FE], f32, tag="scr")
            nc.tensor.matmul(hw_ps[:], jts[:], wwa2_sb[:],
                             start=True, stop=True)
            nc.scalar.copy(h_ext[:, jc, 0:F], hw_ps[:, 0:F])
            nc.vector.tensor_copy(wh2_sb[:, jc:jc + 1], hw_ps[:, F:FE])

        # ---- phase 2: main loop over j windows / j chunks ----------------
        pt_pool = ctx.enter_context(
            tc.tile_pool(name="pt_acc", bufs=1, space="PSUM"))
        pt_ps = pt_pool.tile([FE, no], f32)

        n_act = int(round(lrelu_act_frac * ncj))

        def lrelu_engine(jc):
            # deterministic interleave of ACT / DVE chunks
            return "act" if (jc * 7919) % ncj < n_act else "dve"

        def lrelu_chunk(jc, dst):
            if lrelu_engine(jc) == "act":
                nc.scalar.activation(dst, wh1_rep[:], AF.Prelu,
                                     bias=wh2_sb[:, jc:jc + 1],
                                     scale=1.0, alpha=ALPHA)
            else:
                # t = 0.2 * (Wh1 + Wh2) ; E = max(Wh1 + Wh2, t)
                nc.vector.tensor_scalar(dst, wh1_rep[:],
                                        wh2_sb[:, jc:jc + 1], ALPHA,
                                        AluOpType.add, AluOpType.mult)
                nc.vector.scalar_tensor_tensor(
                    dst, wh1_rep[:], wh2_sb[:, jc:jc + 1], dst,
                    AluOpType.add, AluOpType.max)

        # phase-1b chunks are produced spread over the first nw-1 windows,
        # always ahead of their first use by the main loop.
        per_w = -(-ncj // max(1, nw - 1))
        npair = cpw // 2
        assert cpw % 2 == 0

        with (
            tc.tile_pool(name="adjw", bufs=2 * nic) as adjw_pool,
            tc.tile_pool(name="adjt", bufs=4, space="PSUM") as tr_pool,
            tc.tile_pool(name="epool", bufs=2) as e_pool,
            tc.tile_pool(name="upool", bufs=2 * ns) as u_pool,
        ):
            adjw = {}
            for w in range(nw):
                # adj window DMA (SWDGE cast int32 -> bf16), one per i chunk
                for ic in range(nic):
                    t = adjw_pool.tile([P, jw], bf16, tag="adjw",
                                       name=f"adjw_{w}_{ic}")
                    nc.gpsimd.dma_start(
                        t[:],
                        adj_own[ic * P:(ic + 1) * P, w * jw:(w + 1) * jw])
                    adjw[ic] = t
                wchunks = list(range(w * per_w, min(ncj, (w + 1) * per_w)))
                for jp in range(npair):
                    for k, jc1b in enumerate(wchunks):
                        if k * npair // len(wchunks) == jp:
                            phase1b_chunk(jc1b)
                    jcs = [w * cpw + 2 * jp, w * cpw + 2 * jp + 1]
                    # leakyrelu for both chunks, one batched exp
                    e_sb = e_pool.tile([P, 2, no], f32, tag="e")
                    for q, jc in enumerate(jcs):
                        lrelu_chunk(jc, e_sb[:, q, :])
                    nc.scalar.activation(e_sb[:], e_sb[:], AF.Exp)
                    for q, jc in enumerate(jcs):
                        jcl = jc - w * cpw
                        for s in range(ns):
                            sw = min(S, no - s * S)
                            # adj.T for this i-split: regular bf16 matmuls
                            # against the identity (exact for 0/1) — gets
                            # FWL + weight double-buffering, unlike the
                            # transpose_mode path.
                            at_ps = tr_pool.tile([P, S], f32, tag="adjt")
                            for q2 in range(sw // P):
                                ic = (s * S) // P + q2
                                nc.tensor.matmul(
                                    at_ps[:, q2 * P:(q2 + 1) * P],
                                    adjw[ic][:, jcl * P:(jcl + 1) * P],
                                    identity_bf[:], start=True, stop=True)
                            # U = E * adj.T ; P.T += h_ext.T @ U
                            u_sb = u_pool.tile([P, S], u_dt, tag="u")
                            nc.vector.tensor_tensor(
                                u_sb[:, 0:sw],
                                e_sb[:, q, s * S:s * S + sw],
                                at_ps[:, 0:sw], AluOpType.mult)
                            nc.tensor.matmul(pt_ps[:, s * S:s * S + sw],
                                             h_ext[:, jc, :],
                                             u_sb[:, 0:sw],
                                             start=(jc == 0),
                                             stop=(jc == ncj - 1))

        # ---- phase 3: out = elu(P[:, :64] / P[:, 64]) --------------------
        pt_sb = consts.tile([FE, no], f32)
        nc.vector.tensor_copy(pt_sb[:], pt_ps[:])
        with tc.tile_pool(name="fin_sb", bufs=4) as fin_sb:
            for ic in range(nic):
                ptp = scr_ps.tile([P, FE], f32, tag="scr")
                nc.tensor.transpose(ptp[:], pt_sb[:, ic * P:(ic + 1) * P],
                                    identity[0:FE, 0:FE])
                rec = fin_sb.tile([P, 1], f32, tag="rec")
                nc.vector.reciprocal(rec[:], ptp[:, F:FE])
                hp = fin_sb.tile([P, F], f32, tag="hp")
                nc.vector.tensor_scalar(hp[:], ptp[:, 0:F], rec[:], None,
                                        AluOpType.mult)
                # elu(x) = max(x,0) + exp(min(x,0)) - 1
                mn = fin_sb.tile([P, F], f32, tag="mn")
                nc.vector.tensor_scalar(mn[:], hp[:], 0.0, None, AluOpType.min)
                nc.scalar.activation(mn[:], mn[:], AF.Exp)
                nc.vector.tensor_scalar(hp[:], hp[:], 0.0, None, AluOpType.max)
                ob = fin_sb.tile([P, F], f32, tag="ob")
                nc.vector.scalar_tensor_tensor(
                    ob[:], mn[:], 1.0, hp[:],
                    AluOpType.subtract, AluOpType.add)
                nc.sync.dma_start(out_d[ic * P:(ic + 1) * P, :], ob[:])

    nc.compile()
    return nc


_CACHE = {}


def _get_program(nt, no, jw, **kw):
    key = (nt, no, jw, tuple(sorted(kw.items())))
    if key not in _CACHE:
        _CACHE[key] = build_program(nt, no, jw, **kw)
    return _CACHE[key]


def kernel(input, adj, W, a):
    from concourse.bass_utils import run_bass_kernel_spmd

    input = np.ascontiguousarray(input, dtype=np.float32)
    adj = np.ascontiguousarray(adj, dtype=np.int32)
    W = np.ascontiguousarray(W, dtype=np.float32)
    a = np.ascontiguousarray(a, dtype=np.float32)

    nt = input.shape[0]
    no = nt // N_CORES
    nc = _get_program(nt, no, 2048)

    in_maps = []
    for c in range(N_CORES):
        in_maps.append({
            "input": input,
            "input_own": input[c * no:(c + 1) * no],
            "adj_own": adj[c * no:(c + 1) * no],
            "W": W,
            "a": a,
        })
    res = run_bass_kernel_spmd(nc, in_maps, list(range(N_CORES)))
    return np.concatenate([r["out"] for r in res.results], axis=0)
